# revision 13
# baseline (speedup 1.0000x reference)
"""Multi-head attention (B=4, S=2048, E=768, H=12, Dh=64) on 8 TRN2 NeuronCores.

Sharding: batch x head-group tensor parallel. Core c handles batch b = c//2 and
head group g = c%2 (6 heads each). Each core computes its heads' Q/K/V
projections, full attention over the 2048-token sequence, and a partial
out-projection over its 384 concat-features. The host sums the two partials per
batch and adds the output bias.

Device layout notes (v2 — fp8 scores + dual-engine exp):
 - Host pre-transposes activations to x^T [E, S] and casts to bf16.
 - Q^T/K^T are produced feature-major in fp8e4 (e4m3); the score matmuls run
   in MatmulPerfMode.DoubleRow (0.5 cyc/row): stationary kT tiles carry an
   interleaved zero plane ([64, 2, 128] with plane 1 = 0) so the 2-plane
   contraction reduces over exactly dh=64; the moving qT plane is a stride-0
   broadcast. This halves the PE cost of QK^T.
 - exp(scores) is split within each head-half between the Activation engine
   (native Exp) and the Vector engine via a custom 8-stage DVE op
   EXP16_ATTN_ANT: ((C0*s + C1)^2 + C2)^16, a minimax quadratic for
   exp(s/128) raised to the 16th power (~0.3% max rel err; the common-mode
   part cancels in softmax). This removes the single-engine exp bottleneck.
 - V is token-major bf16, each head augmented with 64 ones columns so the PV
   matmul emits the softmax denominator replicated on psum partitions 64-127
   (normalization is one DVE reciprocal + one multiply-cast). No max
   subtraction: logits are ~N(0, 0.31), exp is numerically safe.
 - PSUM (8 banks): proj 2 (bufs=2 x 1 bank) + double-buffered S^T (4) +
   ctx (2); the PE stream is software-pipelined by hand and projection /
   out-projection work is injected into the exp-bound attention windows.
"""

import math
import os
import sys
from collections import deque
from contextlib import ExitStack

import numpy as np

for _p in ("/opt/trn_rl_repo", "/root/.axon_site/_ro/trn_rl_repo"):
    if os.path.isdir(_p) and _p not in sys.path:
        sys.path.append(_p)

# NTFF tracing hooks (antenv.axon_hooks) don't exist in this container;
# make sure an ambient BASS_TRACE can't route execution into that path.
os.environ["BASS_NEVER_TRACE"] = "1"

import ml_dtypes  # noqa: E402

import concourse.bass as bass  # noqa: E402
import concourse.dve_ops as dve_ops_mod  # noqa: E402
import concourse.tile as tile  # noqa: E402
from concourse import bacc, mybir  # noqa: E402
from concourse.bass_utils import run_bass_kernel_spmd  # noqa: E402
from concourse.dve_ops import DveOp  # noqa: E402
from concourse.dve_spec import C0 as _SC0  # noqa: E402
from concourse.dve_spec import C1 as _SC1  # noqa: E402
from concourse.dve_spec import C2 as _SC2  # noqa: E402
from concourse.dve_spec import Spec, Src0, lower as dve_lower, sq  # noqa: E402
from concourse.dve_uop import DveOpSpec  # noqa: E402

BF16 = mybir.dt.bfloat16
F32 = mybir.dt.float32
F8E4 = mybir.dt.float8e4
NP_BF16 = ml_dtypes.bfloat16
DR = mybir.MatmulPerfMode.DoubleRow

B, S, E, H, DH = 4, 2048, 768, 12, 64
N_CORES = 8
G = H // 2  # heads per core (6)

# exp(s/8) = ((C0*s + C1)^2 + C2)^16: minimax quadratic for e^w on
# |w| <= 2.0/16 (w = s/128), from a relative-error iterated-LSQ fit.
# Design logit range |s/8| <= 2.0 (6.5 sigma); ~2e-3 max rel err after ^16.
EXP_C0 = 0.005523382563715868
EXP_C1 = 0.7097210512400703
EXP_C2 = 0.4963158742836641

# exp(scores) is split column-wise per tile: ACT takes [0:ACT_W), DVE the rest
ACT_W = 768


def _register_exp16() -> DveOp:
    name = "EXP16_ATTN_ANT"
    for op in dve_ops_mod.OPS:
        if op.name == name:
            return op
    body = sq(sq(sq(sq(sq(Src0 * _SC0 + _SC1) + _SC2))))

    def _ref(in0, in1, c0, c1, c2):
        p = (in0.astype(np.float32) * np.float32(c0) + np.float32(c1)).astype(
            np.float32
        )
        p = (p * p + np.float32(c2)).astype(np.float32)
        for _ in range(4):
            p = (p * p).astype(np.float32)
        return p

    spec = Spec(body=body, reference=_ref)
    row = dve_ops_mod._CUSTOM_DVE_ROW_BASE + len(dve_ops_mod.OPS)
    assert row < 0x20, "custom-DVE opcode rows exhausted"
    sha = DveOpSpec(
        name=name, opcode=row, uops=dve_lower(spec, ver="v3"), rd1_en=False
    ).sha("v3")
    op = DveOp(name=name, spec=spec, subdim=False, uops_sha={"v3": sha})
    dve_ops_mod.OPS.append(op)
    dve_ops_mod._SUB_OPCODE_FOR_NAME[name] = row
    dve_ops_mod.CUSTOM_DVE_SPECS[name] = spec
    return op


EXP16_OP = _register_exp16()


def build_nc(T=S, EMB=E, NH=G, dh=DH, OUT=E, trace_label=""):
    """Emit the per-core Bass/Tile program. All cores run this same program.

    T: sequence length; EMB: model dim; NH: heads on this core (even);
    dh: head dim (64); OUT: out-projection output width.
    """
    assert T % 256 == 0 and EMB % 128 == 0 and dh == 64 and NH % 2 == 0
    FEAT = NH * dh
    assert FEAT % 128 == 0
    EC = EMB // 128  # contraction chunks for projections
    TT = T // 128  # token tiles
    FT = FEAT // 128  # feature tiles (head pairs)
    SCH = min(512, T)  # matmul moving free-dim chunk (proj)
    NSCH = T // SCH
    T2 = max(128, T // 2)  # attention query-half width (2 PSUM banks)
    NSH = T // T2  # query halves per head
    SCH2 = min(512, T2)  # PV moving chunk
    NSCH2 = T2 // SCH2
    SCH8 = min(256, T2)  # DoubleRow score moving chunk (2N <= 512)
    NSCH8 = T2 // SCH8
    _ock = OUT // 2 if 128 < OUT <= 1024 and OUT % 2 == 0 else 512
    OCHUNKS = [(o, min(_ock, OUT - o)) for o in range(0, OUT, _ock)]
    scale = 1.0 / math.sqrt(dh)

    nc = bacc.Bacc("TRN2", target_bir_lowering=False, debug=False, num_devices=N_CORES)

    # ---- DRAM I/O ----
    xqT_d = nc.dram_tensor("xqT", [EMB, T], BF16, kind="ExternalInput").ap()
    xkT_d = nc.dram_tensor("xkT", [EMB, T], BF16, kind="ExternalInput").ap()
    xvT_d = nc.dram_tensor("xvT", [EMB, T], BF16, kind="ExternalInput").ap()
    wq_d = nc.dram_tensor("wq", [EMB, FEAT], BF16, kind="ExternalInput").ap()
    wk_d = nc.dram_tensor("wk", [EMB, FEAT], BF16, kind="ExternalInput").ap()
    wv_d = nc.dram_tensor("wv", [EMB, FEAT], BF16, kind="ExternalInput").ap()
    wo_d = nc.dram_tensor("wo", [FEAT, OUT], BF16, kind="ExternalInput").ap()
    bqT_d = nc.dram_tensor("bqT", [128, FT], F32, kind="ExternalInput").ap()
    bkT_d = nc.dram_tensor("bkT", [128, FT], F32, kind="ExternalInput").ap()
    bv_d = nc.dram_tensor("bv", [1, FEAT], BF16, kind="ExternalInput").ap()
    out_d = nc.dram_tensor("out", [T, OUT], BF16, kind="ExternalOutput").ap()

    with tile.TileContext(nc) as tc, ExitStack() as ctx:
        persist = ctx.enter_context(tc.tile_pool(name="persist", bufs=1))

        # ---- persistent SBUF tensors ----
        wq_sb = [persist.tile([128, FEAT], BF16, tag=f"wq{j}", name=f"wq{j}") for j in range(EC)]
        wk_sb = [persist.tile([128, FEAT], BF16, tag=f"wk{j}", name=f"wk{j}") for j in range(EC)]
        wv_sb = [persist.tile([128, FEAT], BF16, tag=f"wv{j}", name=f"wv{j}") for j in range(EC)]
        wo_sb = [persist.tile([128, OUT], BF16, tag=f"wo{j}", name=f"wo{j}") for j in range(FT)]
        # q/k biases transposed to [128, FT] (column j = ftile j, partition =
        # feature) so the ACT proj-copy folds the bias add
        bqT_sb = persist.tile([128, FT], F32, tag="bqT", name="bqT")
        bkT_sb = persist.tile([128, FT], F32, tag="bkT", name="bkT")
        bv_sb = persist.tile([1, FEAT], BF16, tag="bv", name="bv")
        ones_row = persist.tile([1, T], BF16, tag="ones_row", name="ones_row")
        xqT_sb = [persist.tile([128, T], BF16, tag=f"xq{j}", name=f"xq{j}") for j in range(EC)]
        xkT_sb = [persist.tile([128, T], BF16, tag=f"xk{j}", name=f"xk{j}") for j in range(EC)]
        xvT_sb = [persist.tile([128, T], BF16, tag=f"xv{j}", name=f"xv{j}") for j in range(EC)]
        # q feature-major fp8; k feature-major fp8 with interleaved zero
        # planes: per key-tile i, cols [i*256, i*256+128) = data, rest zeros
        qT_sb = [persist.tile([128, T], F8E4, tag=f"qT{j}", name=f"qT{j}") for j in range(FT)]
        kT_sb = [persist.tile([128, 2 * T], F8E4, tag=f"kT{j}", name=f"kT{j}") for j in range(FT)]
        # V token-major, each head augmented with 64 ones columns so the PV
        # matmul emits the softmax denominator replicated on partitions 64-127
        v_sb = [persist.tile([128, NH * (dh + 64)], BF16, tag=f"v{i}", name=f"v{i}") for i in range(TT)]
        cn_sb = [persist.tile([128, T], BF16, tag=f"cn{j}", name=f"cn{j}") for j in range(FT)]

        # ---- weight/bias/x loads (Q/K path first: it gates head 0) ----
        nc.sync.dma_start(bqT_sb[:], bqT_d[:])
        nc.sync.dma_start(bkT_sb[:], bkT_d[:])
        for j in range(EC):
            nc.sync.dma_start(wq_sb[j][:], wq_d[j * 128 : (j + 1) * 128, :])
            nc.sync.dma_start(xqT_sb[j][:], xqT_d[j * 128 : (j + 1) * 128, :])
            nc.sync.dma_start(wk_sb[j][:], wk_d[j * 128 : (j + 1) * 128, :])
            nc.sync.dma_start(xkT_sb[j][:], xkT_d[j * 128 : (j + 1) * 128, :])
        nc.sync.dma_start(bv_sb[:], bv_d[:])
        for j in range(EC):
            nc.sync.dma_start(wv_sb[j][:], wv_d[j * 128 : (j + 1) * 128, :])
            nc.sync.dma_start(xvT_sb[j][:], xvT_d[j * 128 : (j + 1) * 128, :])
        for j in range(FT):
            nc.sync.dma_start(wo_sb[j][:], wo_d[j * 128 : (j + 1) * 128, :])
        nc.vector.memset(ones_row[:], 1.0)
        # zero planes of kT (plane 1 of each [64, 2, 128] stationary)
        for j in range(FT):
            kz = kT_sb[j][:].rearrange("p (i two x) -> p i two x", two=2, x=128)
            nc.gpsimd.memset(kz[:, :, 1, :], 0.0)
        # ones columns of augmented V (written once)
        for i in range(TT):
            vview = v_sb[i][:].rearrange("p (h x) -> p h x", x=dh + 64)
            nc.gpsimd.memset(vview[:, :, dh:], 1.0)

        # ---- compute: projections + attention + out-projection ----
        # PSUM budget (8 banks): proj 2 (bufs=2 x 1 bank) + ST 4 (bufs=2 x 2)
        # + ctx 2 (bufs=1 x 2). Everything coexists, so Tile can overlap the
        # phases; PE instruction order is software-pipelined by hand.
        with (
            tc.tile_pool(name="ppsum", bufs=2, space="PSUM") as ppool,
            tc.tile_pool(name="stpsum", bufs=2, space="PSUM") as stpool,
            tc.tile_pool(name="ctpsum", bufs=1, space="PSUM") as ctpool,
            tc.tile_pool(name="ptpool", bufs=5) as ptpool,
            tc.tile_pool(name="normpool", bufs=3) as npool,
            tc.tile_pool(name="outsb", bufs=4) as osbpool,
        ):

            def qcopy(j, n, ps_ap):
                nc.vector.tensor_scalar_add(
                    qT_sb[j][:, n * SCH : (n + 1) * SCH],
                    ps_ap,
                    bqT_sb[:, j : j + 1],
                )

            def kcopy(j, n, ps_ap):
                # scatter the 512-key chunk into the zero-interleaved layout
                kv = kT_sb[j][:].rearrange("p (i two x) -> p i two x", two=2, x=128)
                t0 = n * SCH // 128
                dst = kv[:, t0 : t0 + SCH // 128, 0, :]
                srcv = ps_ap.rearrange("p (i x) -> p i x", x=128)
                nc.vector.tensor_scalar_add(dst, srcv, bkT_sb[:, j : j + 1])

            qk_tabs = (
                (wq_sb, xqT_sb, qcopy),
                (wk_sb, xkT_sb, kcopy),
            )

            # ---- uniform PE filler pump: all projection / out-projection
            # work is chopped into ~0.3-0.5us quanta dispensed one(ish) per
            # attention tile-beat, so PE never idles during exp-bound windows
            # and never lumps enough work to stall the score pipeline.
            fillq = deque()
            _acc = [0.0]

            def pump(rate):
                _acc[0] += rate
                n = int(_acc[0])
                if n > 0 and fillq:
                    _acc[0] -= n
                for _ in range(n):
                    if fillq:
                        fillq.popleft()()

            def drain_fillq():
                while fillq:
                    fillq.popleft()()

            def enq_proj_qk(j):
                for t in range(2):
                    for n in range(NSCH):
                        hold = {}

                        def mk(e0, t=t, n=n, hold=hold, j=j):
                            def f():
                                if e0 == 0:
                                    hold["ps"] = ppool.tile(
                                        [128, SCH], F32, tag="proj", name="proj"
                                    )
                                w_sb, x_sb, _ = qk_tabs[t]
                                for e in (e0, e0 + 1):
                                    nc.tensor.matmul(
                                        hold["ps"][:],
                                        w_sb[e][:, j * 128 : (j + 1) * 128],
                                        x_sb[e][:, n * SCH : (n + 1) * SCH],
                                        start=(e == 0),
                                        stop=(e == EC - 1),
                                    )

                            return f

                        def cp(t=t, n=n, hold=hold, j=j):
                            def f():
                                qk_tabs[t][2](j, n, hold["ps"][:])

                            return f

                        for e0 in range(0, EC, 2):
                            fillq.append(mk(e0))
                        fillq.append(cp())

            def enq_outproj(i):
                assert FT == 3
                hold = {}

                def q1(oc, ow, first):
                    def f():
                        if first:
                            hold["osb"] = osbpool.tile(
                                [128, OUT], BF16, tag="osb", name="osb"
                            )
                        hold[oc] = ppool.tile([128, ow], F32, tag="proj", name="proj")
                        for ff in (0, 1):
                            nc.tensor.matmul(
                                hold[oc][:],
                                cn_sb[ff][:, i * 128 : (i + 1) * 128],
                                wo_sb[ff][:, oc : oc + ow],
                                start=(ff == 0),
                                stop=False,
                            )

                    return f

                def q2(oc, ow):
                    def f():
                        nc.tensor.matmul(
                            hold[oc][:],
                            cn_sb[2][:, i * 128 : (i + 1) * 128],
                            wo_sb[2][:, oc : oc + ow],
                            start=False,
                            stop=True,
                        )
                        nc.vector.tensor_copy(hold["osb"][:, oc : oc + ow], hold[oc][:])

                    return f

                def dma():
                    def f():
                        nc.sync.dma_start(
                            out_d[i * 128 : (i + 1) * 128, :], hold["osb"][:]
                        )

                    return f

                for ci, (oc, ow) in enumerate(OCHUNKS):
                    fillq.append(q1(oc, ow, ci == 0))
                    fillq.append(q2(oc, ow))
                fillq.append(dma())

            def proj_v(tiles=None):
                for i in tiles if tiles is not None else range(TT):
                    ps = ppool.tile([128, FEAT], F32, tag="proj", name="proj")
                    nc.tensor.matmul(
                        ps[:], ones_row[:, 0:128], bv_sb[:], start=True, stop=False
                    )
                    for e in range(EC):
                        nc.tensor.matmul(
                            ps[:],
                            xvT_sb[e][:, i * 128 : (i + 1) * 128],
                            wv_sb[e][:],
                            start=False,
                            stop=(e == EC - 1),
                        )
                    dst = v_sb[i][:].rearrange("p (h x) -> p h x", x=dh + 64)[:, :, 0:dh]
                    srcv = ps[:].rearrange("p (h d) -> p h d", d=dh)
                    nc.vector.tensor_copy(dst, srcv)

            def st_tile(i, ft, hb, s0):
                # fp8 DoubleRow: lhsT [64, 2, 128] (plane 1 = zeros), moving
                # qT [64, 2, N] via stride-0 broadcast; out [128 keys, N]
                st = stpool.tile([128, T2], F32, tag="st", name="st")
                kv = kT_sb[ft][:].rearrange("p (i two x) -> p i two x", two=2, x=128)
                lhs = kv[hb : hb + 64, i]
                for n in range(NSCH8):
                    rhs = (
                        qT_sb[ft][hb : hb + 64, s0 + n * SCH8 : s0 + (n + 1) * SCH8]
                        .unsqueeze(1)
                        .broadcast_to([64, 2, SCH8])
                    )
                    nc.tensor.matmul(
                        st[:, n * SCH8 : (n + 1) * SCH8],
                        lhs,
                        rhs,
                        start=True,
                        stop=True,
                        perf_mode=DR,
                    )
                return st

            pending_sts = []

            def head_args(h, sh):
                return (h // 2, (h % 2) * 64, sh * T2)

            def head(h, sh, filler=None, nxt=None, act_w=None, rate=1.0):
                # keeps 2 score tiles in flight and pre-issues the NEXT
                # head's first 2 before this head's last context matmul, so
                # the exp stream never stalls at head boundaries
                ft, hb, s0 = head_args(h, sh)
                aw = min(act_w or ACT_W, T2)
                ct = ctpool.tile([128, T2], F32, tag="ct", name="ct")
                sts = pending_sts[:]
                del pending_sts[:]
                while len(sts) < min(2, TT):
                    sts.append(st_tile(len(sts), ft, hb, s0))
                nissued = 0
                for i in range(TT):
                    st = sts.pop(0)
                    pt = ptpool.tile([128, T2], BF16, tag="pt", name="pt")
                    nc.scalar.activation(
                        pt[:, 0:aw],
                        st[:, 0:aw],
                        mybir.ActivationFunctionType.Exp,
                        scale=scale,
                    )
                    if aw < T2:
                        nc.vector._custom_dve(
                            EXP16_OP,
                            out=pt[:, aw:T2],
                            in0=st[:, aw:T2],
                            s0=EXP_C0,
                            s1=EXP_C1,
                            imm2=EXP_C2,
                        )
                    # PE filler first: it is dependency-free, so it runs while
                    # the engines exp tile i
                    if filler is not None:
                        filler(i)
                    else:
                        pump(rate)
                    if i + 2 < TT:
                        sts.append(st_tile(i + 2, ft, hb, s0))
                    elif nxt is not None and nissued < min(2, TT):
                        pending_sts.append(st_tile(nissued, *head_args(*nxt)))
                        nissued += 1
                    for n in range(NSCH2):
                        nc.tensor.matmul(
                            ct[:, n * SCH2 : (n + 1) * SCH2],
                            v_sb[i][:, h * (dh + 64) : (h + 1) * (dh + 64)],
                            pt[:, n * SCH2 : (n + 1) * SCH2],
                            start=(i == 0),
                            stop=(i == TT - 1),
                        )

                # normalize: cn[f, s] = ct[f, s] * (1 / ct[64.., s])
                recip = npool.tile([64, T2], F32, tag="recip", name="recip")
                for c0 in range(0, T2, 512):
                    c1 = min(c0 + 512, T2)
                    nc.vector.reciprocal(recip[:, c0:c1], ct[64:128, c0:c1])
                    nc.vector.tensor_tensor(
                        cn_sb[ft][hb : hb + 64, s0 + c0 : s0 + c1],
                        ct[0:64, c0:c1],
                        recip[:, c0:c1],
                        op=mybir.AluOpType.mult,
                    )

            # ---- warmup: pair-0 Q/K projection, E-outer across 8 PSUM
            # accumulation groups so PE starts as soon as the first x/w DMA
            # tiles land instead of after the whole stream
            assert NSCH == 4 and EC % 2 == 0
            st_w = [stpool.tile([128, T2], F32, tag="st", name="st") for _ in range(2)]
            ct_w = ctpool.tile([128, T2], F32, tag="ct", name="ct")
            pp_w = [ppool.tile([128, SCH], F32, tag="proj", name="proj") for _ in range(2)]
            qdst = [
                st_w[0][:, 0:512], st_w[0][:, 512:1024],
                st_w[1][:, 0:512], st_w[1][:, 512:1024],
            ]
            kdst = [ct_w[:, 0:512], ct_w[:, 512:1024], pp_w[0][:], pp_w[1][:]]
            for e in range(EC):
                for n in range(NSCH):
                    nc.tensor.matmul(
                        qdst[n],
                        wq_sb[e][:, 0:128],
                        xqT_sb[e][:, n * SCH : (n + 1) * SCH],
                        start=(e == 0),
                        stop=(e == EC - 1),
                    )
                for n in range(NSCH):
                    nc.tensor.matmul(
                        kdst[n],
                        wk_sb[e][:, 0:128],
                        xkT_sb[e][:, n * SCH : (n + 1) * SCH],
                        start=(e == 0),
                        stop=(e == EC - 1),
                    )
            for n in range(NSCH):
                kcopy(0, n, kdst[n])
            for n in range(NSCH):
                qcopy(0, n, qdst[n])

            # pre-issue head 0's first score tiles BEFORE any V work: V
            # depends on the last-arriving xvT DMAs and must not gate exp_0
            for z in range(min(2, TT)):
                pending_sts.append(st_tile(z, *head_args(0, 0)))
            # V tile i is first needed at head 0's CT step i: emit tile 0/1
            # up front and drip the rest into head 0's pipeline
            proj_v(range(2))

            def v_filler(i):
                if i + 2 < TT:
                    proj_v([i + 2])

            half_tiles = T2 // 128 if NSH == 2 else 0
            assert NSH == 2 and NH == 6
            # sh-outer unit order: all query-half-0 heads, then all half-1
            seq = [(h, sh) for sh in range(NSH) for h in range(NH)]
            # per-unit (act_w, pump_rate): filler-rich units lean on ACT for
            # exp; filler-poor units shift exp toward the DVE
            unit_cfg = {
                0: (768, 0.0),
                1: (768, 2.5),
                2: (768, 1.5),
                3: (768, 1.0),
                4: (672, 1.0),
                5: (672, 1.0),
            }
            for u in range(6, 12):
                unit_cfg[u] = (704, 0.5)
            for u, (h, sh) in enumerate(seq):
                if u == 1:
                    enq_proj_qk(1)
                elif u == 2:
                    enq_proj_qk(2)
                elif u == 6:
                    for i in range(half_tiles):
                        enq_outproj(i)
                aw, rate = unit_cfg[u]
                nxt = seq[u + 1] if u + 1 < len(seq) else None
                head(
                    h,
                    sh,
                    v_filler if u == 0 else None,
                    nxt=nxt,
                    act_w=aw,
                    rate=rate,
                )
            drain_fillq()
            for i in range(half_tiles, TT):
                enq_outproj(i)
            drain_fillq()

    nc.compile()
    return nc


def shard_inputs(query, key, value, wq, bq, wk, bk, wv, bv, wo):
    """Build the 8 per-core input maps (host-side cast/transpose/slice)."""
    in_maps = []
    xT = {}
    for b in range(B):
        xT[b] = (
            np.ascontiguousarray(query[b].T).astype(NP_BF16),
            np.ascontiguousarray(key[b].T).astype(NP_BF16),
            np.ascontiguousarray(value[b].T).astype(NP_BF16),
        )
    gw = {}
    for g in range(2):
        hs = slice(g * G, (g + 1) * G)
        gw[g] = dict(
            wq=np.ascontiguousarray(wq[hs].transpose(1, 0, 2).reshape(E, G * DH)).astype(NP_BF16),
            wk=np.ascontiguousarray(wk[hs].transpose(1, 0, 2).reshape(E, G * DH)).astype(NP_BF16),
            wv=np.ascontiguousarray(wv[hs].transpose(1, 0, 2).reshape(E, G * DH)).astype(NP_BF16),
            wo=np.ascontiguousarray(wo[g * G * DH : (g + 1) * G * DH, :]).astype(NP_BF16),
            bqT=np.ascontiguousarray(
                bq[hs].reshape(G * DH // 128, 128).T
            ).astype(np.float32),
            bkT=np.ascontiguousarray(
                bk[hs].reshape(G * DH // 128, 128).T
            ).astype(np.float32),
            bv=np.ascontiguousarray(bv[hs].reshape(1, G * DH)).astype(NP_BF16),
        )
    for c in range(N_CORES):
        b, g = c // 2, c % 2
        m = dict(xqT=xT[b][0], xkT=xT[b][1], xvT=xT[b][2])
        m.update(gw[g])
        in_maps.append(m)
    return in_maps


_CACHED_NC = None


def kernel(query, key, value, wq, bq, wk, bk, wv, bv, wo, bo):
    global _CACHED_NC
    query, key, value = (np.asarray(a, np.float32) for a in (query, key, value))
    wq, bq, wk, bk, wv, bv, wo, bo = (
        np.asarray(a, np.float32) for a in (wq, bq, wk, bk, wv, bv, wo, bo)
    )
    in_maps = shard_inputs(query, key, value, wq, bq, wk, bk, wv, bv, wo)
    if _CACHED_NC is None:
        _CACHED_NC = build_nc()
    res = run_bass_kernel_spmd(_CACHED_NC, in_maps, list(range(N_CORES)))
    out = np.empty((B, S, E), np.float32)
    for b in range(B):
        out[b] = (
            res.results[2 * b]["out"].astype(np.float32)
            + res.results[2 * b + 1]["out"].astype(np.float32)
            + bo[None, :]
        )
    return out


# revision 14
# speedup vs baseline: 1.0067x; 1.0067x over previous
"""Multi-head attention (B=4, S=2048, E=768, H=12, Dh=64) on 8 TRN2 NeuronCores.

Sharding: batch x head-group tensor parallel. Core c handles batch b = c//2 and
head group g = c%2 (6 heads each). Each core computes its heads' Q/K/V
projections, full attention over the 2048-token sequence, and a partial
out-projection over its 384 concat-features. The host sums the two partials per
batch and adds the output bias.

Device layout notes (v2 — fp8 scores + dual-engine exp):
 - Host pre-transposes activations to x^T [E, S] and casts to bf16.
 - Q^T/K^T are produced feature-major in fp8e4 (e4m3); the score matmuls run
   in MatmulPerfMode.DoubleRow (0.5 cyc/row): stationary kT tiles carry an
   interleaved zero plane ([64, 2, 128] with plane 1 = 0) so the 2-plane
   contraction reduces over exactly dh=64; the moving qT plane is a stride-0
   broadcast. This halves the PE cost of QK^T.
 - exp(scores) is split within each head-half between the Activation engine
   (native Exp) and the Vector engine via a custom 8-stage DVE op
   EXP16_ATTN_ANT: ((C0*s + C1)^2 + C2)^16, a minimax quadratic for
   exp(s/128) raised to the 16th power (~0.3% max rel err; the common-mode
   part cancels in softmax). This removes the single-engine exp bottleneck.
 - V is token-major bf16, each head augmented with 64 ones columns so the PV
   matmul emits the softmax denominator replicated on psum partitions 64-127
   (normalization is one DVE reciprocal + one multiply-cast). No max
   subtraction: logits are ~N(0, 0.31), exp is numerically safe.
 - PSUM (8 banks): proj 2 (bufs=2 x 1 bank) + double-buffered S^T (4) +
   ctx (2); the PE stream is software-pipelined by hand and projection /
   out-projection work is injected into the exp-bound attention windows.
"""

import math
import os
import sys
from collections import deque
from contextlib import ExitStack

import numpy as np

for _p in ("/opt/trn_rl_repo", "/root/.axon_site/_ro/trn_rl_repo"):
    if os.path.isdir(_p) and _p not in sys.path:
        sys.path.append(_p)

# NTFF tracing hooks (antenv.axon_hooks) don't exist in this container;
# make sure an ambient BASS_TRACE can't route execution into that path.
os.environ["BASS_NEVER_TRACE"] = "1"

import ml_dtypes  # noqa: E402

import concourse.bass as bass  # noqa: E402
import concourse.dve_ops as dve_ops_mod  # noqa: E402
import concourse.tile as tile  # noqa: E402
from concourse import bacc, mybir  # noqa: E402
from concourse.bass_utils import run_bass_kernel_spmd  # noqa: E402
from concourse.dve_ops import DveOp  # noqa: E402
from concourse.dve_spec import C0 as _SC0  # noqa: E402
from concourse.dve_spec import C1 as _SC1  # noqa: E402
from concourse.dve_spec import C2 as _SC2  # noqa: E402
from concourse.dve_spec import Spec, Src0, lower as dve_lower, sq  # noqa: E402
from concourse.dve_uop import DveOpSpec  # noqa: E402

BF16 = mybir.dt.bfloat16
F32 = mybir.dt.float32
F8E4 = mybir.dt.float8e4
NP_BF16 = ml_dtypes.bfloat16
DR = mybir.MatmulPerfMode.DoubleRow

B, S, E, H, DH = 4, 2048, 768, 12, 64
N_CORES = 8
G = H // 2  # heads per core (6)

# exp(s/8) = ((C0*s + C1)^2 + C2)^16: minimax quadratic for e^w on
# |w| <= 2.0/16 (w = s/128), from a relative-error iterated-LSQ fit.
# Design logit range |s/8| <= 2.0 (6.5 sigma); ~2e-3 max rel err after ^16.
EXP_C0 = 0.005523382563715868
EXP_C1 = 0.7097210512400703
EXP_C2 = 0.4963158742836641

# exp(scores) is split column-wise per tile: ACT takes [0:ACT_W), DVE the rest
ACT_W = 768


def _register_exp16() -> DveOp:
    name = "EXP16_ATTN_ANT"
    for op in dve_ops_mod.OPS:
        if op.name == name:
            return op
    body = sq(sq(sq(sq(sq(Src0 * _SC0 + _SC1) + _SC2))))

    def _ref(in0, in1, c0, c1, c2):
        p = (in0.astype(np.float32) * np.float32(c0) + np.float32(c1)).astype(
            np.float32
        )
        p = (p * p + np.float32(c2)).astype(np.float32)
        for _ in range(4):
            p = (p * p).astype(np.float32)
        return p

    spec = Spec(body=body, reference=_ref)
    row = dve_ops_mod._CUSTOM_DVE_ROW_BASE + len(dve_ops_mod.OPS)
    assert row < 0x20, "custom-DVE opcode rows exhausted"
    sha = DveOpSpec(
        name=name, opcode=row, uops=dve_lower(spec, ver="v3"), rd1_en=False
    ).sha("v3")
    op = DveOp(name=name, spec=spec, subdim=False, uops_sha={"v3": sha})
    dve_ops_mod.OPS.append(op)
    dve_ops_mod._SUB_OPCODE_FOR_NAME[name] = row
    dve_ops_mod.CUSTOM_DVE_SPECS[name] = spec
    return op


EXP16_OP = _register_exp16()


def build_nc(T=S, EMB=E, NH=G, dh=DH, OUT=E, trace_label=""):
    """Emit the per-core Bass/Tile program. All cores run this same program.

    T: sequence length; EMB: model dim; NH: heads on this core (even);
    dh: head dim (64); OUT: out-projection output width.
    """
    assert T % 256 == 0 and EMB % 128 == 0 and dh == 64 and NH % 2 == 0
    FEAT = NH * dh
    assert FEAT % 128 == 0
    EC = EMB // 128  # contraction chunks for projections
    TT = T // 128  # token tiles
    FT = FEAT // 128  # feature tiles (head pairs)
    SCH = min(512, T)  # matmul moving free-dim chunk (proj)
    NSCH = T // SCH
    T2 = max(128, T // 2)  # attention query-half width (2 PSUM banks)
    NSH = T // T2  # query halves per head
    SCH2 = min(512, T2)  # PV moving chunk
    NSCH2 = T2 // SCH2
    SCH8 = min(256, T2)  # DoubleRow score moving chunk (2N <= 512)
    NSCH8 = T2 // SCH8
    _ock = OUT // 2 if 128 < OUT <= 1024 and OUT % 2 == 0 else 512
    OCHUNKS = [(o, min(_ock, OUT - o)) for o in range(0, OUT, _ock)]
    scale = 1.0 / math.sqrt(dh)

    nc = bacc.Bacc("TRN2", target_bir_lowering=False, debug=False, num_devices=N_CORES)

    # ---- DRAM I/O ----
    xqT_d = nc.dram_tensor("xqT", [EMB, T], BF16, kind="ExternalInput").ap()
    xkT_d = nc.dram_tensor("xkT", [EMB, T], BF16, kind="ExternalInput").ap()
    xvT_d = nc.dram_tensor("xvT", [EMB, T], BF16, kind="ExternalInput").ap()
    wq_d = nc.dram_tensor("wq", [EMB, FEAT], BF16, kind="ExternalInput").ap()
    wk_d = nc.dram_tensor("wk", [EMB, FEAT], BF16, kind="ExternalInput").ap()
    wv_d = nc.dram_tensor("wv", [EMB, FEAT], BF16, kind="ExternalInput").ap()
    wo_d = nc.dram_tensor("wo", [FEAT, OUT], BF16, kind="ExternalInput").ap()
    bqT_d = nc.dram_tensor("bqT", [128, FT], F32, kind="ExternalInput").ap()
    bkT_d = nc.dram_tensor("bkT", [128, FT], F32, kind="ExternalInput").ap()
    bv_d = nc.dram_tensor("bv", [1, FEAT], BF16, kind="ExternalInput").ap()
    out_d = nc.dram_tensor("out", [T, OUT], BF16, kind="ExternalOutput").ap()

    with tile.TileContext(nc) as tc, ExitStack() as ctx:
        persist = ctx.enter_context(tc.tile_pool(name="persist", bufs=1))

        # ---- persistent SBUF tensors ----
        wq_sb = [persist.tile([128, FEAT], BF16, tag=f"wq{j}", name=f"wq{j}") for j in range(EC)]
        wk_sb = [persist.tile([128, FEAT], BF16, tag=f"wk{j}", name=f"wk{j}") for j in range(EC)]
        wv_sb = [persist.tile([128, FEAT], BF16, tag=f"wv{j}", name=f"wv{j}") for j in range(EC)]
        wo_sb = [persist.tile([128, OUT], BF16, tag=f"wo{j}", name=f"wo{j}") for j in range(FT)]
        # q/k biases transposed to [128, FT] (column j = ftile j, partition =
        # feature) so the ACT proj-copy folds the bias add
        bqT_sb = persist.tile([128, FT], F32, tag="bqT", name="bqT")
        bkT_sb = persist.tile([128, FT], F32, tag="bkT", name="bkT")
        bv_sb = persist.tile([1, FEAT], BF16, tag="bv", name="bv")
        ones_row = persist.tile([1, T], BF16, tag="ones_row", name="ones_row")
        xqT_sb = [persist.tile([128, T], BF16, tag=f"xq{j}", name=f"xq{j}") for j in range(EC)]
        xkT_sb = [persist.tile([128, T], BF16, tag=f"xk{j}", name=f"xk{j}") for j in range(EC)]
        xvT_sb = [persist.tile([128, T], BF16, tag=f"xv{j}", name=f"xv{j}") for j in range(EC)]
        # q feature-major fp8; k feature-major fp8 with interleaved zero
        # planes: per key-tile i, cols [i*256, i*256+128) = data, rest zeros
        qT_sb = [persist.tile([128, T], F8E4, tag=f"qT{j}", name=f"qT{j}") for j in range(FT)]
        kT_sb = [persist.tile([128, 2 * T], F8E4, tag=f"kT{j}", name=f"kT{j}") for j in range(FT)]
        # V token-major, each head augmented with 64 ones columns so the PV
        # matmul emits the softmax denominator replicated on partitions 64-127
        v_sb = [persist.tile([128, NH * (dh + 64)], BF16, tag=f"v{i}", name=f"v{i}") for i in range(TT)]
        cn_sb = [persist.tile([128, T], BF16, tag=f"cn{j}", name=f"cn{j}") for j in range(FT)]

        # ---- weight/bias/x loads (Q/K path first: it gates head 0) ----
        nc.sync.dma_start(bqT_sb[:], bqT_d[:])
        nc.sync.dma_start(bkT_sb[:], bkT_d[:])
        for j in range(EC):
            nc.sync.dma_start(wq_sb[j][:], wq_d[j * 128 : (j + 1) * 128, :])
            nc.sync.dma_start(xqT_sb[j][:], xqT_d[j * 128 : (j + 1) * 128, :])
            nc.sync.dma_start(wk_sb[j][:], wk_d[j * 128 : (j + 1) * 128, :])
            nc.sync.dma_start(xkT_sb[j][:], xkT_d[j * 128 : (j + 1) * 128, :])
        nc.sync.dma_start(bv_sb[:], bv_d[:])
        for j in range(EC):
            nc.sync.dma_start(wv_sb[j][:], wv_d[j * 128 : (j + 1) * 128, :])
            nc.sync.dma_start(xvT_sb[j][:], xvT_d[j * 128 : (j + 1) * 128, :])
        for j in range(FT):
            nc.sync.dma_start(wo_sb[j][:], wo_d[j * 128 : (j + 1) * 128, :])
        nc.vector.memset(ones_row[:], 1.0)
        # zero planes of kT (plane 1 of each [64, 2, 128] stationary)
        for j in range(FT):
            kz = kT_sb[j][:].rearrange("p (i two x) -> p i two x", two=2, x=128)
            nc.gpsimd.memset(kz[:, :, 1, :], 0.0)
        # ones columns of augmented V (written once)
        for i in range(TT):
            vview = v_sb[i][:].rearrange("p (h x) -> p h x", x=dh + 64)
            nc.gpsimd.memset(vview[:, :, dh:], 1.0)

        # ---- compute: projections + attention + out-projection ----
        # PSUM budget (8 banks): proj 2 (bufs=2 x 1 bank) + ST 4 (bufs=2 x 2)
        # + ctx 2 (bufs=1 x 2). Everything coexists, so Tile can overlap the
        # phases; PE instruction order is software-pipelined by hand.
        with (
            tc.tile_pool(name="ppsum", bufs=2, space="PSUM") as ppool,
            tc.tile_pool(name="stpsum", bufs=2, space="PSUM") as stpool,
            tc.tile_pool(name="ctpsum", bufs=1, space="PSUM") as ctpool,
            tc.tile_pool(name="ptpool", bufs=5) as ptpool,
            tc.tile_pool(name="normpool", bufs=3) as npool,
            tc.tile_pool(name="outsb", bufs=4) as osbpool,
        ):

            def qcopy(j, n, ps_ap):
                nc.vector.tensor_scalar_add(
                    qT_sb[j][:, n * SCH : (n + 1) * SCH],
                    ps_ap,
                    bqT_sb[:, j : j + 1],
                )

            def kcopy(j, n, ps_ap):
                # scatter the 512-key chunk into the zero-interleaved layout
                kv = kT_sb[j][:].rearrange("p (i two x) -> p i two x", two=2, x=128)
                t0 = n * SCH // 128
                dst = kv[:, t0 : t0 + SCH // 128, 0, :]
                srcv = ps_ap.rearrange("p (i x) -> p i x", x=128)
                nc.vector.tensor_scalar_add(dst, srcv, bkT_sb[:, j : j + 1])

            qk_tabs = (
                (wq_sb, xqT_sb, qcopy),
                (wk_sb, xkT_sb, kcopy),
            )

            # ---- uniform PE filler pump: all projection / out-projection
            # work is chopped into ~0.3-0.5us quanta dispensed one(ish) per
            # attention tile-beat, so PE never idles during exp-bound windows
            # and never lumps enough work to stall the score pipeline.
            fillq = deque()
            _acc = [0.0]

            def pump(rate):
                _acc[0] += rate
                n = int(_acc[0])
                if n > 0 and fillq:
                    _acc[0] -= n
                for _ in range(n):
                    if fillq:
                        fillq.popleft()()

            def drain_fillq():
                while fillq:
                    fillq.popleft()()

            def enq_proj_qk(j):
                for t in range(2):
                    for n in range(NSCH):
                        hold = {}

                        def mk(e0, t=t, n=n, hold=hold, j=j):
                            def f():
                                if e0 == 0:
                                    hold["ps"] = ppool.tile(
                                        [128, SCH], F32, tag="proj", name="proj"
                                    )
                                w_sb, x_sb, _ = qk_tabs[t]
                                for e in (e0, e0 + 1):
                                    nc.tensor.matmul(
                                        hold["ps"][:],
                                        w_sb[e][:, j * 128 : (j + 1) * 128],
                                        x_sb[e][:, n * SCH : (n + 1) * SCH],
                                        start=(e == 0),
                                        stop=(e == EC - 1),
                                    )

                            return f

                        def cp(t=t, n=n, hold=hold, j=j):
                            def f():
                                qk_tabs[t][2](j, n, hold["ps"][:])

                            return f

                        for e0 in range(0, EC, 2):
                            fillq.append(mk(e0))
                        fillq.append(cp())

            def enq_outproj(i):
                assert FT == 3
                hold = {}

                def q1(oc, ow, first):
                    def f():
                        if first:
                            hold["osb"] = osbpool.tile(
                                [128, OUT], BF16, tag="osb", name="osb"
                            )
                        hold[oc] = ppool.tile([128, ow], F32, tag="proj", name="proj")
                        for ff in (0, 1):
                            nc.tensor.matmul(
                                hold[oc][:],
                                cn_sb[ff][:, i * 128 : (i + 1) * 128],
                                wo_sb[ff][:, oc : oc + ow],
                                start=(ff == 0),
                                stop=False,
                            )

                    return f

                def q2(oc, ow):
                    def f():
                        nc.tensor.matmul(
                            hold[oc][:],
                            cn_sb[2][:, i * 128 : (i + 1) * 128],
                            wo_sb[2][:, oc : oc + ow],
                            start=False,
                            stop=True,
                        )
                        nc.vector.tensor_copy(hold["osb"][:, oc : oc + ow], hold[oc][:])

                    return f

                def dma():
                    def f():
                        nc.sync.dma_start(
                            out_d[i * 128 : (i + 1) * 128, :], hold["osb"][:]
                        )

                    return f

                for ci, (oc, ow) in enumerate(OCHUNKS):
                    fillq.append(q1(oc, ow, ci == 0))
                    fillq.append(q2(oc, ow))
                fillq.append(dma())

            def proj_v(tiles=None):
                for i in tiles if tiles is not None else range(TT):
                    ps = ppool.tile([128, FEAT], F32, tag="proj", name="proj")
                    nc.tensor.matmul(
                        ps[:], ones_row[:, 0:128], bv_sb[:], start=True, stop=False
                    )
                    for e in range(EC):
                        nc.tensor.matmul(
                            ps[:],
                            xvT_sb[e][:, i * 128 : (i + 1) * 128],
                            wv_sb[e][:],
                            start=False,
                            stop=(e == EC - 1),
                        )
                    dst = v_sb[i][:].rearrange("p (h x) -> p h x", x=dh + 64)[:, :, 0:dh]
                    srcv = ps[:].rearrange("p (h d) -> p h d", d=dh)
                    nc.vector.tensor_copy(dst, srcv)

            def st_tile(i, ft, hb, s0):
                # fp8 DoubleRow: lhsT [64, 2, 128] (plane 1 = zeros), moving
                # qT [64, 2, N] via stride-0 broadcast; out [128 keys, N]
                st = stpool.tile([128, T2], F32, tag="st", name="st")
                kv = kT_sb[ft][:].rearrange("p (i two x) -> p i two x", two=2, x=128)
                lhs = kv[hb : hb + 64, i]
                for n in range(NSCH8):
                    rhs = (
                        qT_sb[ft][hb : hb + 64, s0 + n * SCH8 : s0 + (n + 1) * SCH8]
                        .unsqueeze(1)
                        .broadcast_to([64, 2, SCH8])
                    )
                    nc.tensor.matmul(
                        st[:, n * SCH8 : (n + 1) * SCH8],
                        lhs,
                        rhs,
                        start=True,
                        stop=True,
                        perf_mode=DR,
                    )
                return st

            pending_sts = []

            def head_args(h, sh):
                return (h // 2, (h % 2) * 64, sh * T2)

            def head(h, sh, filler=None, nxt=None, act_w=None, rate=1.0):
                # keeps 2 score tiles in flight and pre-issues the NEXT
                # head's first 2 before this head's last context matmul, so
                # the exp stream never stalls at head boundaries.
                # Tiles 0-7 lean hard on ACT so the DVE queue has slack to
                # absorb the previous head's normalize + pump copies without
                # delaying the st-buffer WAR chain; tiles 8-15 rebalance.
                ft, hb, s0 = head_args(h, sh)
                ct = ctpool.tile([128, T2], F32, tag="ct", name="ct")
                sts = pending_sts[:]
                del pending_sts[:]
                while len(sts) < min(2, TT):
                    sts.append(st_tile(len(sts), ft, hb, s0))
                nissued = 0
                for i in range(TT):
                    aw = 896 if i < TT // 2 else 704
                    st = sts.pop(0)
                    pt = ptpool.tile([128, T2], BF16, tag="pt", name="pt")
                    nc.scalar.activation(
                        pt[:, 0:aw],
                        st[:, 0:aw],
                        mybir.ActivationFunctionType.Exp,
                        scale=scale,
                    )
                    if aw < T2:
                        nc.vector._custom_dve(
                            EXP16_OP,
                            out=pt[:, aw:T2],
                            in0=st[:, aw:T2],
                            s0=EXP_C0,
                            s1=EXP_C1,
                            imm2=EXP_C2,
                        )
                    # PE filler first: it is dependency-free, so it runs while
                    # the engines exp tile i
                    if filler is not None:
                        filler(i)
                    else:
                        pump(rate)
                    if i + 2 < TT:
                        sts.append(st_tile(i + 2, ft, hb, s0))
                    elif nxt is not None and nissued < min(2, TT):
                        pending_sts.append(st_tile(nissued, *head_args(*nxt)))
                        nissued += 1
                    for n in range(NSCH2):
                        nc.tensor.matmul(
                            ct[:, n * SCH2 : (n + 1) * SCH2],
                            v_sb[i][:, h * (dh + 64) : (h + 1) * (dh + 64)],
                            pt[:, n * SCH2 : (n + 1) * SCH2],
                            start=(i == 0),
                            stop=(i == TT - 1),
                        )

                # normalize: cn[f, s] = ct[f, s] * (1 / ct[64.., s])
                recip = npool.tile([64, T2], F32, tag="recip", name="recip")
                for c0 in range(0, T2, 512):
                    c1 = min(c0 + 512, T2)
                    nc.vector.reciprocal(recip[:, c0:c1], ct[64:128, c0:c1])
                    nc.vector.tensor_tensor(
                        cn_sb[ft][hb : hb + 64, s0 + c0 : s0 + c1],
                        ct[0:64, c0:c1],
                        recip[:, c0:c1],
                        op=mybir.AluOpType.mult,
                    )

            # ---- warmup: pair-0 Q/K projection, E-outer across 8 PSUM
            # accumulation groups so PE starts as soon as the first x/w DMA
            # tiles land instead of after the whole stream
            assert NSCH == 4 and EC % 2 == 0
            st_w = [stpool.tile([128, T2], F32, tag="st", name="st") for _ in range(2)]
            ct_w = ctpool.tile([128, T2], F32, tag="ct", name="ct")
            pp_w = [ppool.tile([128, SCH], F32, tag="proj", name="proj") for _ in range(2)]
            qdst = [
                st_w[0][:, 0:512], st_w[0][:, 512:1024],
                st_w[1][:, 0:512], st_w[1][:, 512:1024],
            ]
            kdst = [ct_w[:, 0:512], ct_w[:, 512:1024], pp_w[0][:], pp_w[1][:]]
            for e in range(EC):
                for n in range(NSCH):
                    nc.tensor.matmul(
                        qdst[n],
                        wq_sb[e][:, 0:128],
                        xqT_sb[e][:, n * SCH : (n + 1) * SCH],
                        start=(e == 0),
                        stop=(e == EC - 1),
                    )
                for n in range(NSCH):
                    nc.tensor.matmul(
                        kdst[n],
                        wk_sb[e][:, 0:128],
                        xkT_sb[e][:, n * SCH : (n + 1) * SCH],
                        start=(e == 0),
                        stop=(e == EC - 1),
                    )
            for n in range(NSCH):
                kcopy(0, n, kdst[n])
            for n in range(NSCH):
                qcopy(0, n, qdst[n])

            # pre-issue head 0's first score tiles BEFORE any V work: V
            # depends on the last-arriving xvT DMAs and must not gate exp_0
            for z in range(min(2, TT)):
                pending_sts.append(st_tile(z, *head_args(0, 0)))
            # V tile i is first needed at head 0's CT step i: emit tile 0/1
            # up front and drip the rest into head 0's pipeline
            proj_v(range(2))

            def v_filler(i):
                if i + 2 < TT:
                    proj_v([i + 2])

            half_tiles = T2 // 128 if NSH == 2 else 0
            assert NSH == 2 and NH == 6
            # sh-outer unit order: all query-half-0 heads, then all half-1
            seq = [(h, sh) for sh in range(NSH) for h in range(NH)]
            # per-unit (act_w, pump_rate): filler-rich units lean on ACT for
            # exp; filler-poor units shift exp toward the DVE
            unit_cfg = {
                0: (768, 0.0),
                1: (768, 2.5),
                2: (768, 1.5),
                3: (768, 1.0),
                4: (672, 1.0),
                5: (672, 1.0),
            }
            for u in range(6, 12):
                unit_cfg[u] = (704, 0.5)
            for u, (h, sh) in enumerate(seq):
                if u == 1:
                    enq_proj_qk(1)
                elif u == 2:
                    enq_proj_qk(2)
                elif u == 6:
                    for i in range(half_tiles):
                        enq_outproj(i)
                aw, rate = unit_cfg[u]
                nxt = seq[u + 1] if u + 1 < len(seq) else None
                head(
                    h,
                    sh,
                    v_filler if u == 0 else None,
                    nxt=nxt,
                    act_w=aw,
                    rate=rate,
                )
            drain_fillq()
            for i in range(half_tiles, TT):
                enq_outproj(i)
            drain_fillq()

    nc.compile()
    return nc


def shard_inputs(query, key, value, wq, bq, wk, bk, wv, bv, wo):
    """Build the 8 per-core input maps (host-side cast/transpose/slice)."""
    in_maps = []
    xT = {}
    for b in range(B):
        xT[b] = (
            np.ascontiguousarray(query[b].T).astype(NP_BF16),
            np.ascontiguousarray(key[b].T).astype(NP_BF16),
            np.ascontiguousarray(value[b].T).astype(NP_BF16),
        )
    gw = {}
    for g in range(2):
        hs = slice(g * G, (g + 1) * G)
        gw[g] = dict(
            wq=np.ascontiguousarray(wq[hs].transpose(1, 0, 2).reshape(E, G * DH)).astype(NP_BF16),
            wk=np.ascontiguousarray(wk[hs].transpose(1, 0, 2).reshape(E, G * DH)).astype(NP_BF16),
            wv=np.ascontiguousarray(wv[hs].transpose(1, 0, 2).reshape(E, G * DH)).astype(NP_BF16),
            wo=np.ascontiguousarray(wo[g * G * DH : (g + 1) * G * DH, :]).astype(NP_BF16),
            bqT=np.ascontiguousarray(
                bq[hs].reshape(G * DH // 128, 128).T
            ).astype(np.float32),
            bkT=np.ascontiguousarray(
                bk[hs].reshape(G * DH // 128, 128).T
            ).astype(np.float32),
            bv=np.ascontiguousarray(bv[hs].reshape(1, G * DH)).astype(NP_BF16),
        )
    for c in range(N_CORES):
        b, g = c // 2, c % 2
        m = dict(xqT=xT[b][0], xkT=xT[b][1], xvT=xT[b][2])
        m.update(gw[g])
        in_maps.append(m)
    return in_maps


_CACHED_NC = None


def kernel(query, key, value, wq, bq, wk, bk, wv, bv, wo, bo):
    global _CACHED_NC
    query, key, value = (np.asarray(a, np.float32) for a in (query, key, value))
    wq, bq, wk, bk, wv, bv, wo, bo = (
        np.asarray(a, np.float32) for a in (wq, bq, wk, bk, wv, bv, wo, bo)
    )
    in_maps = shard_inputs(query, key, value, wq, bq, wk, bk, wv, bv, wo)
    if _CACHED_NC is None:
        _CACHED_NC = build_nc()
    res = run_bass_kernel_spmd(_CACHED_NC, in_maps, list(range(N_CORES)))
    out = np.empty((B, S, E), np.float32)
    for b in range(B):
        out[b] = (
            res.results[2 * b]["out"].astype(np.float32)
            + res.results[2 * b + 1]["out"].astype(np.float32)
            + bo[None, :]
        )
    return out
